# revision 9
# baseline (speedup 1.0000x reference)
"""2-layer GAT (graph attention) on 8 Trainium2 NeuronCores.

Strategy (dst-partitioned, per the 1D graph partitioning scheme):
 - Node tables: G1 = [f=x@W1 | el | er] for all nodes, G2 likewise for layer 1.
   Each core projects a strip of nodes, then AllGather -> full table on
   every core.
 - Per core, dst nodes are split into 128-row blocks. Edges are bucketed by
   (dst block, src-row range) on the host; per-edge rows of the node table
   are fetched with dma_gather (int16 indices force <=32768-row ranges).
 - Edge softmax: ee = exp(leaky_relu(el[src]+er[dst])); aggregation is a
   one-hot matmul: psum[dst,:] += S.T @ [ee*f | ee], where S[e,d]=1{dloc[e]==d}
   is built on-chip with is_equal against an iota row. er[dst] per edge is
   produced without a gather: er_edge = St.T @ er_block where St is the
   transposed one-hot (built from a partition-broadcast copy of dloc).
 - Normalization (divide by sum ee), bias, relu in the block epilogue; the
   layer-2 projection consumes the PE-transposed h tile immediately, so h
   never round-trips through DRAM.
 - dst assignment is chosen so each core's layer-1 dst rows are a prefix of
   its own layer-0 dst rows (er2 comes from the core's own layer-0 epilogue,
   keeping the program SPMD-uniform).
"""

import math
import numpy as np

P = 128

CFG = dict(
    NC=8,
    N0=100000, N1=50000, N2=25000,
    H1=4, D1=64, H2=1, D2=64,
    F0=256, SLOPE=0.2,
    RANGE=32768,
)


def _ceil_to(x, m):
    return -(-x // m) * m


def _derive(cfg):
    NC = cfg["NC"]
    d = {}
    d["n0pc"] = _ceil_to(cfg["N0"], NC * P) // NC
    d["N0P"] = NC * d["n0pc"]
    d["n2pc"] = _ceil_to(cfg["N2"], NC * P) // NC
    d["N2P"] = NC * d["n2pc"]
    rem = cfg["N1"] - d["N2P"]
    assert rem > 0, "layout assumes N1 > padded N2"
    d["bpc"] = _ceil_to(rem, NC * P) // NC
    d["n1pc"] = d["n2pc"] + d["bpc"]
    d["N1P"] = NC * d["n1pc"]
    d["nb0"] = d["n1pc"] // P      # layer-0 dst blocks per core
    d["nb1"] = d["n2pc"] // P      # layer-1 dst blocks per core
    d["nr0"] = -(-d["N0P"] // cfg["RANGE"])
    d["nr1"] = -(-d["N1P"] // cfg["RANGE"])
    d["TW1"] = cfg["F0"] + 64      # 256 f + 4 el + 4 er + pad -> 320 (1280B)
    d["PJ1"] = cfg["F0"] + 2 * cfg["H1"]   # 264 projected cols
    d["TW2"] = 128                 # 64 f2 + el2 + er2 + pad -> 128 (512B)
    d["PJ2"] = cfg["D2"] + 2   # 66: [f2 | el2 | er2]
    d["mk1"] = cfg["F0"] // P      # K chunks for layer-0 projection (2)
    d["mk2"] = (cfg["H1"] * cfg["D1"]) // P  # K chunks for layer-1 proj (2)
    return d


def _l0_owner_local(dst, d):
    """layer-0 dst node -> (core, local row). A-part = first n2pc rows of each
    core (aligned with the layer-1 dst range), B-part = the rest."""
    n2pc, bpc, N2P = d["n2pc"], d["bpc"], d["N2P"]
    a = dst < N2P
    c = np.where(a, dst // n2pc, (dst - N2P) // bpc)
    loc = np.where(a, dst % n2pc, n2pc + (dst - N2P) % bpc)
    return c.astype(np.int64), loc.astype(np.int64)


def _prep_edges(src_rows, dst_c, dst_loc, nb, nr, rng_size, NC):
    """Bucket edges by (core, block, range); pad each bucket to a multiple of
    128 slots, chunk counts maxed across cores (SPMD needs one program).

    Returns (segments, totals, per_core) where
      segments: list of (b, r, local_chunk0, nch, idxcol0) shared by all cores
      totals:   (total_chunks, chunk_base[nb])
      per_core: list of dicts with idx16 [128, S/16], dloc_col [128, C],
                dloc_bc [1, S]
    """
    blk = dst_loc // P
    dl = (dst_loc % P).astype(np.float32)
    rng = src_rows // rng_size
    sl = src_rows % rng_size
    assert sl.max(initial=0) < 32768

    counts = np.zeros((NC, nb, nr), np.int64)
    np.add.at(counts, (dst_c, blk, rng), 1)
    maxch = -(-counts.max(axis=0) // P)          # [nb, nr]
    Cb = maxch.sum(axis=1)                       # [nb]
    chunk_base = np.concatenate([[0], np.cumsum(Cb)])
    total_chunks = int(Cb.sum())
    total_slots = total_chunks * P

    # shared segment table + idx16 column offsets
    segments = []
    idxcol = 0
    seg_info = {}
    for b in range(nb):
        lc = 0
        for r in range(nr):
            nch = int(maxch[b, r])
            if nch == 0:
                continue
            assert nch * P <= 1024, f"gather call too big: {nch * P}"
            segments.append((b, r, lc, nch, idxcol))
            seg_info[(b, r)] = (lc, nch, idxcol)
            lc += nch
            idxcol += nch * 8          # nch*128/16 int16 columns
        assert lc == Cb[b]

    order = np.lexsort((rng, blk, dst_c))
    key = (dst_c * nb + blk) * nr + rng
    skey = key[order]
    bounds = np.searchsorted(skey, np.arange(NC * nb * nr + 1))

    per_core = []
    for c in range(NC):
        idx16 = np.zeros((16, total_slots // 16), np.int16)
        dloc_col = np.full((P, total_chunks), 999.0, np.float32)
        dloc_bc = np.full(total_slots, 999.0, np.float32)
        for b in range(nb):
            for r in range(nr):
                if (b, r) not in seg_info:
                    continue
                lc, nch, col0 = seg_info[(b, r)]
                k = (c * nb + b) * nr + r
                e = order[bounds[k]:bounds[k + 1]]
                n = len(e)
                nslot = nch * P
                assert n <= nslot
                sidx = np.zeros(nslot, np.int64)
                sidx[:n] = sl[e]
                sdl = np.full(nslot, 999.0, np.float32)
                sdl[:n] = dl[e]
                idx16[:, col0:col0 + nch * 8] = (
                    sidx.reshape(-1, 16).T.astype(np.int16))
                g0 = chunk_base[b] + lc
                dloc_col[:, g0:g0 + nch] = sdl.reshape(nch, P).T
                dloc_bc[g0 * P:(g0 + nch) * P] = sdl
        per_core.append(dict(
            idx16=np.tile(idx16, (8, 1)),
            dloc_col=dloc_col,
            dloc_bc=dloc_bc[None, :],
        ))
    return segments, (total_chunks, chunk_base), per_core


def _host_prep(inputs, cfg):
    d = _derive(cfg)
    NC = cfg["NC"]
    F0, H1, D1, D2 = cfg["F0"], cfg["H1"], cfg["D1"], cfg["D2"]

    x = np.asarray(inputs["x"], np.float32)
    xp = np.zeros((d["N0P"], F0), np.float32)
    xp[:cfg["N0"]] = x

    # weight packing: G1 cols = [f | el | er], same for layer 2
    al1 = np.asarray(inputs["al1"], np.float32)
    ar1 = np.asarray(inputs["ar1"], np.float32)
    A_l = np.zeros((H1 * D1, H1), np.float32)
    A_r = np.zeros((H1 * D1, H1), np.float32)
    for h in range(H1):
        A_l[h * D1:(h + 1) * D1, h] = al1[h]
        A_r[h * D1:(h + 1) * D1, h] = ar1[h]
    W1 = np.asarray(inputs["W1"], np.float32)
    W1e = np.concatenate([W1, W1 @ A_l, W1 @ A_r], axis=1)  # [F0, PJ1]

    W2 = np.asarray(inputs["W2"], np.float32)
    al2 = np.asarray(inputs["al2"], np.float32).reshape(-1, 1)
    ar2 = np.asarray(inputs["ar2"], np.float32).reshape(-1, 1)
    W2e = np.concatenate([W2, W2 @ al2, W2 @ ar2], axis=1)  # [256, 66]

    # edges, layer 0: table row of src = src node id
    e0s = np.asarray(inputs["e0_src"], np.int64)
    e0d = np.asarray(inputs["e0_dst"], np.int64)
    c0, loc0 = _l0_owner_local(e0d, d)
    seg0, tot0, pc0 = _prep_edges(e0s, c0, loc0, d["nb0"], d["nr0"],
                                  cfg["RANGE"], NC)

    # edges, layer 1: table row of src node n = owner0(n)*n1pc + local0(n)
    e1s = np.asarray(inputs["e1_src"], np.int64)
    e1d = np.asarray(inputs["e1_dst"], np.int64)
    sc, sloc = _l0_owner_local(e1s, d)
    g2row = sc * d["n1pc"] + sloc
    c1 = e1d // d["n2pc"]
    loc1 = e1d % d["n2pc"]
    seg1, tot1, pc1 = _prep_edges(g2row, c1, loc1, d["nb1"], d["nr1"],
                                  cfg["RANGE"], NC)

    b1 = np.asarray(inputs["b1"], np.float32)
    b2 = np.asarray(inputs["b2"], np.float32)

    consts = dict(
        W1e_a=W1e[:P].copy(), W1e_b=W1e[P:].copy(),
        W2e_a=W2e[:P].copy(), W2e_b=W2e[P:].copy(),
        IOTA_ROW=np.tile(np.arange(P, dtype=np.float32), (P, 1)).copy(),
        IOTA_COL=np.arange(P, dtype=np.float32)[:, None].copy(),
        IDENT=np.eye(P, dtype=np.float32),
        B1T=np.tile(b1[None, :], (P, 1)).copy(),
        B2T=np.tile(b2[None, :], (P, 1)).copy(),
    )

    in_maps = []
    for c in range(NC):
        xT = np.ascontiguousarray(xp[c * d["n0pc"]:(c + 1) * d["n0pc"]].T)
        rows_a = np.arange(c * d["n2pc"], (c + 1) * d["n2pc"])
        rows_b = d["N2P"] + np.arange(c * d["bpc"], (c + 1) * d["bpc"])
        xTd = np.ascontiguousarray(xp[np.concatenate([rows_a, rows_b])].T)
        m = dict(consts)
        m["xT"] = xT
        m["xTd"] = xTd
        m["IDX0"] = pc0[c]["idx16"]
        m["DLC0"] = pc0[c]["dloc_col"]
        m["DLB0"] = pc0[c]["dloc_bc"]
        m["IDX1"] = pc1[c]["idx16"]
        m["DLC1"] = pc1[c]["dloc_col"]
        m["DLB1"] = pc1[c]["dloc_bc"]
        in_maps.append(m)

    meta = dict(d=d, seg0=seg0, tot0=tot0, seg1=seg1, tot1=tot1)
    return in_maps, meta


def _build(cfg, meta, stages="all"):
    import concourse.bass as bass
    import concourse.bacc as bacc
    import concourse.mybir as mybir
    import concourse.tile as tile

    d = meta["d"]
    NC = cfg["NC"]
    F0, H1, D1, D2 = cfg["F0"], cfg["H1"], cfg["D1"], cfg["D2"]
    SLOPE = cfg["SLOPE"]
    TW1, PJ1, TW2, PJ2 = d["TW1"], d["PJ1"], d["TW2"], d["PJ2"]
    nb0, nb1 = d["nb0"], d["nb1"]
    seg0, (C0, cb0) = meta["seg0"], meta["tot0"]
    seg1, (C1, cb1) = meta["seg1"], meta["tot1"]
    S0, S1 = C0 * P, C1 * P
    fdt = mybir.dt.float32
    AL = mybir.AluOpType

    nc = bacc.Bacc("TRN2", target_bir_lowering=False, debug=False,
                   num_devices=NC)

    def din(name, shape, dt=fdt):
        return nc.dram_tensor(name, shape, dt, kind="ExternalInput")

    xT = din("xT", [F0, d["n0pc"]])
    xTd = din("xTd", [F0, d["n1pc"]])
    W1e_a = din("W1e_a", [P, PJ1]); W1e_b = din("W1e_b", [P, PJ1])
    W2e_a = din("W2e_a", [P, PJ2]); W2e_b = din("W2e_b", [P, PJ2])
    IOTA_ROW = din("IOTA_ROW", [P, P]); IOTA_COL = din("IOTA_COL", [P, 1])
    IDENT = din("IDENT", [P, P])
    B1T = din("B1T", [P, F0]); B2T = din("B2T", [P, D2])
    IDX0 = din("IDX0", [P, S0 // 16], mybir.dt.int16)
    DLC0 = din("DLC0", [P, C0]); DLB0 = din("DLB0", [1, S0])
    IDX1 = din("IDX1", [P, S1 // 16], mybir.dt.int16)
    DLC1 = din("DLC1", [P, C1]); DLB1 = din("DLB1", [1, S1])
    OUT = nc.dram_tensor("OUT", [d["n2pc"], D2], fdt, kind="ExternalOutput")

    G1S = nc.dram_tensor("G1S", [d["n0pc"], TW1], fdt)
    G1F = nc.dram_tensor("G1F", [d["N0P"], TW1], fdt, addr_space="Shared")
    G2S = nc.dram_tensor("G2S", [d["n1pc"], TW2], fdt)
    G2F = nc.dram_tensor("G2F", [d["N1P"], TW2], fdt, addr_space="Shared")

    def bcast_row(dram, s0, n):
        """DRAM [1, N] slice -> AP broadcast across 128 partitions."""
        ap = dram[0:1, s0:s0 + n]
        return bass.AP(ap.tensor, ap.offset, [[0, P], [1, n]])

    seg_by_block0 = {}
    for (b, r, lc, nch, col0) in seg0:
        seg_by_block0.setdefault(b, []).append((r, lc, nch, col0))
    seg_by_block1 = {}
    for (b, r, lc, nch, col0) in seg1:
        seg_by_block1.setdefault(b, []).append((r, lc, nch, col0))

    with tile.TileContext(nc) as tc:
        with tc.tile_pool(name="const", bufs=1) as cp, \
             tc.tile_pool(name="work", bufs=2) as wp, \
             tc.tile_pool(name="chk", bufs=4) as kp, \
             tc.tile_pool(name="psum", bufs=2, space="PSUM") as pp, \
             tc.tile_pool(name="psE", bufs=2, space="PSUM") as pe, \
             tc.tile_pool(name="psT", bufs=3, space="PSUM") as pt:

            def const_tile(name, dram, shape, dt=fdt):
                t = cp.tile(shape, dt, tag=name)
                nc.sync.dma_start(out=t[:], in_=dram[:, :])
                return t

            w1a = const_tile("w1a", W1e_a, [P, PJ1])
            w1b = const_tile("w1b", W1e_b, [P, PJ1])
            w2a = const_tile("w2a", W2e_a, [P, PJ2])
            w2b = const_tile("w2b", W2e_b, [P, PJ2])
            iorow = const_tile("iorow", IOTA_ROW, [P, P])
            iocol = const_tile("iocol", IOTA_COL, [P, 1])
            ident = const_tile("ident", IDENT, [P, P])
            b1t = const_tile("b1t", B1T, [P, F0])
            b2t = const_tile("b2t", B2T, [P, D2])
            idx0 = const_tile("idx0", IDX0, [P, S0 // 16], mybir.dt.int16)
            dlc0 = const_tile("dlc0", DLC0, [P, C0])
            idx1 = const_tile("idx1", IDX1, [P, S1 // 16], mybir.dt.int16)
            dlc1 = const_tile("dlc1", DLC1, [P, C1])
            er_sb = cp.tile([P, nb0, H1], fdt, tag="er_sb")
            er2_sb = cp.tile([P, nb1, 1], fdt, tag="er2_sb")

            # ---- layer-0 projection: G1 strip = [f | el | er] ----
            for m in range(d["n0pc"] // P):
                ps = pp.tile([P, PJ1], fdt, tag="agg", space="PSUM")
                for k in range(d["mk1"]):
                    xt = kp.tile([P, P], fdt, tag="xt")
                    nc.sync.dma_start(
                        out=xt[:],
                        in_=xT[k * P:(k + 1) * P, m * P:(m + 1) * P])
                    nc.tensor.matmul(out=ps[:], lhsT=xt[:],
                                     rhs=(w1a if k == 0 else w1b)[:],
                                     start=(k == 0), stop=(k == d["mk1"] - 1))
                sb = kp.tile([P, TW1], fdt, tag="pjsb")
                nc.scalar.copy(out=sb[:, 0:PJ1], in_=ps[:])
                nc.vector.memset(sb[:, PJ1:TW1], 0.0)
                nc.sync.dma_start(out=G1S[m * P:(m + 1) * P, :], in_=sb[:])

            # ---- own-dst er projection (kept in SBUF) ----
            for b in range(nb0):
                ps = pe.tile([P, H1], fdt, tag="er", space="PSUM")
                for k in range(d["mk1"]):
                    xt = kp.tile([P, P], fdt, tag="xt")
                    nc.sync.dma_start(
                        out=xt[:],
                        in_=xTd[k * P:(k + 1) * P, b * P:(b + 1) * P])
                    w = (w1a if k == 0 else w1b)
                    nc.tensor.matmul(out=ps[:], lhsT=xt[:],
                                     rhs=w[:, F0 + H1:F0 + 2 * H1],
                                     start=(k == 0), stop=(k == d["mk1"] - 1))
                nc.scalar.copy(out=er_sb[:, b, :], in_=ps[:])

            nc.gpsimd.collective_compute(
                "AllGather", AL.bypass,
                replica_groups=[list(range(NC))],
                ins=[G1S[:, :]], outs=[G1F[:, :]])

            # ---- layer-0 blocks ----
            for b in range(nb0 if stages != "proj" else 0):
                segs = seg_by_block0.get(b, [])
                Cb = sum(nch for (_, _, nch, _) in segs)
                if Cb == 0:
                    # still must produce zero h -> g2 row
                    Cb = 0
                R = wp.tile([P, max(Cb, 1), TW1], fdt, tag="R")
                for (r, lc, nch, col0) in segs:
                    lo = r * cfg["RANGE"]
                    hi = min(lo + cfg["RANGE"], d["N0P"])
                    nsl = nch * P
                    nc.gpsimd.dma_gather(
                        R[:, lc:lc + nch, :], G1F[lo:hi, :],
                        idx0[:, col0:col0 + nch * 8], nsl, nsl, TW1)
                dlb = wp.tile([P, max(Cb, 1) * P], fdt, tag="dlb")
                if Cb:
                    nc.sync.dma_start(
                        out=dlb[:, :Cb * P],
                        in_=bcast_row(DLB0, int(cb0[b]) * P, Cb * P))
                ps = pp.tile([P, PJ1], fdt, tag="agg", space="PSUM")
                for ci in range(Cb):
                    gc = int(cb0[b]) + ci
                    st = kp.tile([P, P], fdt, tag="st")
                    nc.vector.tensor_tensor(
                        out=st[:], in0=iocol[:].to_broadcast([P, P]),
                        in1=dlb[:, ci * P:(ci + 1) * P], op=AL.is_equal)
                    ere = pe.tile([P, H1], fdt, tag="er", space="PSUM")
                    nc.tensor.matmul(out=ere[:], lhsT=st[:],
                                     rhs=er_sb[:, b, :], start=True, stop=True)
                    ee = kp.tile([P, H1], fdt, tag="ee")
                    nc.vector.tensor_tensor(out=ee[:], in0=R[:, ci, F0:F0 + H1],
                                            in1=ere[:], op=AL.add)
                    e2 = kp.tile([P, H1], fdt, tag="e2")
                    nc.vector.tensor_scalar(out=e2[:], in0=ee[:],
                                            scalar1=SLOPE, scalar2=None,
                                            op0=AL.mult)
                    nc.vector.tensor_tensor(out=e2[:], in0=ee[:], in1=e2[:],
                                            op=AL.max)
                    nc.scalar.activation(out=e2[:], in_=e2[:],
                                         func=mybir.ActivationFunctionType.Exp)
                    msg = kp.tile([P, F0 + H1], fdt, tag="msg")
                    for h in range(H1):
                        nc.vector.tensor_scalar(
                            out=msg[:, h * D1:(h + 1) * D1],
                            in0=R[:, ci, h * D1:(h + 1) * D1],
                            scalar1=e2[:, h:h + 1], scalar2=None, op0=AL.mult)
                    nc.scalar.copy(out=msg[:, F0:F0 + H1], in_=e2[:])
                    s = kp.tile([P, P], fdt, tag="s")
                    nc.vector.tensor_tensor(
                        out=s[:], in0=dlc0[:, gc:gc + 1].to_broadcast([P, P]),
                        in1=iorow[:], op=AL.is_equal)
                    nc.tensor.matmul(out=ps[:, 0:F0 + H1], lhsT=s[:],
                                     rhs=msg[:], start=(ci == 0),
                                     stop=(ci == Cb - 1))
                if Cb == 0:
                    z = kp.tile([P, F0 + H1], fdt, tag="msg")
                    nc.vector.memset(z[:], 0.0)
                    s = kp.tile([P, P], fdt, tag="s")
                    nc.vector.memset(s[:], 0.0)
                    nc.tensor.matmul(out=ps[:, 0:F0 + H1], lhsT=s[:],
                                     rhs=z[:], start=True, stop=True)
                # epilogue: normalize, bias, relu
                rr = kp.tile([P, H1], fdt, tag="rr")
                nc.vector.tensor_scalar(out=rr[:], in0=ps[:, F0:F0 + H1],
                                        scalar1=1e-30, scalar2=None, op0=AL.add)
                nc.vector.reciprocal(out=rr[:], in_=rr[:])
                hsb = wp.tile([P, F0], fdt, tag="hsb")
                for h in range(H1):
                    nc.vector.tensor_scalar(
                        out=hsb[:, h * D1:(h + 1) * D1],
                        in0=ps[:, h * D1:(h + 1) * D1],
                        scalar1=rr[:, h:h + 1], scalar2=None, op0=AL.mult)
                nc.vector.tensor_tensor(out=hsb[:], in0=hsb[:], in1=b1t[:],
                                        op=AL.add)
                nc.scalar.activation(out=hsb[:], in_=hsb[:],
                                     func=mybir.ActivationFunctionType.Relu)
                g2p = pt.tile([P, PJ2], fdt, tag="epi", space="PSUM")
                for k in range(d["mk2"]):
                    tp = pt.tile([P, P], fdt, tag="epi", space="PSUM")
                    nc.tensor.transpose(out=tp[:],
                                        in_=hsb[:, k * P:(k + 1) * P],
                                        identity=ident[:])
                    hT = kp.tile([P, P], fdt, tag="hT")
                    nc.scalar.copy(out=hT[:], in_=tp[:])
                    nc.tensor.matmul(out=g2p[:], lhsT=hT[:],
                                     rhs=(w2a if k == 0 else w2b)[:],
                                     start=(k == 0), stop=(k == d["mk2"] - 1))
                g2sb = kp.tile([P, TW2], fdt, tag="g2sb")
                nc.scalar.copy(out=g2sb[:, 0:PJ2], in_=g2p[:])
                nc.vector.memset(g2sb[:, PJ2:TW2], 0.0)
                nc.sync.dma_start(out=G2S[b * P:(b + 1) * P, :], in_=g2sb[:])
                if b < nb1:
                    nc.scalar.copy(out=er2_sb[:, b, :],
                                   in_=g2sb[:, PJ2 - 1:PJ2])

            if stages in ("all", "nol1g"):
                nc.gpsimd.collective_compute(
                    "AllGather", AL.bypass,
                    replica_groups=[list(range(NC))],
                    ins=[G2S[:, :]], outs=[G2F[:, :]])

            # ---- layer-1 blocks ----
            for b in range(nb1 if stages in ("all", "nol1g") else 0):
                segs = seg_by_block1.get(b, [])
                Cb = sum(nch for (_, _, nch, _) in segs)
                R = wp.tile([P, max(Cb, 1), TW2], fdt, tag="R2")
                if stages == "nol1g":
                    nc.vector.memset(R[:], 0.0)
                else:
                    for (r, lc, nch, col0) in segs:
                        lo = r * cfg["RANGE"]
                        hi = min(lo + cfg["RANGE"], d["N1P"])
                        nsl = nch * P
                        nc.gpsimd.dma_gather(
                            R[:, lc:lc + nch, :], G2F[lo:hi, :],
                            idx1[:, col0:col0 + nch * 8], nsl, nsl, TW2)
                dlb = wp.tile([P, max(Cb, 1) * P], fdt, tag="dlb2")
                if Cb:
                    nc.sync.dma_start(
                        out=dlb[:, :Cb * P],
                        in_=bcast_row(DLB1, int(cb1[b]) * P, Cb * P))
                ps = pp.tile([P, D2 + 1], fdt, tag="agg", space="PSUM")
                for ci in range(Cb):
                    gc = int(cb1[b]) + ci
                    st = kp.tile([P, P], fdt, tag="st")
                    nc.vector.tensor_tensor(
                        out=st[:], in0=iocol[:].to_broadcast([P, P]),
                        in1=dlb[:, ci * P:(ci + 1) * P], op=AL.is_equal)
                    ere = pe.tile([P, 1], fdt, tag="er", space="PSUM")
                    nc.tensor.matmul(out=ere[:], lhsT=st[:],
                                     rhs=er2_sb[:, b, :], start=True, stop=True)
                    ee = kp.tile([P, 1], fdt, tag="ee")
                    nc.vector.tensor_tensor(out=ee[:], in0=R[:, ci, D2:D2 + 1],
                                            in1=ere[:], op=AL.add)
                    e2 = kp.tile([P, 1], fdt, tag="e2")
                    nc.vector.tensor_scalar(out=e2[:], in0=ee[:],
                                            scalar1=SLOPE, scalar2=None,
                                            op0=AL.mult)
                    nc.vector.tensor_tensor(out=e2[:], in0=ee[:], in1=e2[:],
                                            op=AL.max)
                    nc.scalar.activation(out=e2[:], in_=e2[:],
                                         func=mybir.ActivationFunctionType.Exp)
                    msg = kp.tile([P, D2 + 1], fdt, tag="msg2")
                    nc.vector.tensor_scalar(out=msg[:, 0:D2],
                                            in0=R[:, ci, 0:D2],
                                            scalar1=e2[:, 0:1], scalar2=None,
                                            op0=AL.mult)
                    nc.scalar.copy(out=msg[:, D2:D2 + 1], in_=e2[:])
                    s = kp.tile([P, P], fdt, tag="s")
                    nc.vector.tensor_tensor(
                        out=s[:], in0=dlc1[:, gc:gc + 1].to_broadcast([P, P]),
                        in1=iorow[:], op=AL.is_equal)
                    nc.tensor.matmul(out=ps[:], lhsT=s[:], rhs=msg[:],
                                     start=(ci == 0), stop=(ci == Cb - 1))
                if Cb == 0:
                    z = kp.tile([P, D2 + 1], fdt, tag="msg2")
                    nc.vector.memset(z[:], 0.0)
                    s = kp.tile([P, P], fdt, tag="s")
                    nc.vector.memset(s[:], 0.0)
                    nc.tensor.matmul(out=ps[:], lhsT=s[:], rhs=z[:],
                                     start=True, stop=True)
                rr = kp.tile([P, 1], fdt, tag="rr")
                nc.vector.tensor_scalar(out=rr[:], in0=ps[:, D2:D2 + 1],
                                        scalar1=1e-30, scalar2=None, op0=AL.add)
                nc.vector.reciprocal(out=rr[:], in_=rr[:])
                osb = kp.tile([P, D2], fdt, tag="osb")
                nc.vector.tensor_scalar(out=osb[:], in0=ps[:, 0:D2],
                                        scalar1=rr[:, 0:1], scalar2=None,
                                        op0=AL.mult)
                nc.vector.tensor_tensor(out=osb[:], in0=osb[:], in1=b2t[:],
                                        op=AL.add)
                nc.sync.dma_start(out=OUT[b * P:(b + 1) * P, :], in_=osb[:])

    nc.compile()
    return nc


def kernel(**inputs):
    from concourse import bass_utils
    cfg = CFG
    in_maps, meta = _host_prep(inputs, cfg)
    nc = _build(cfg, meta)
    res = bass_utils.run_bass_kernel_spmd(
        nc, in_maps, core_ids=list(range(cfg["NC"])))
    d = meta["d"]
    out = np.concatenate([res.results[c]["OUT"] for c in range(cfg["NC"])],
                         axis=0)
    return np.ascontiguousarray(out[:cfg["N2"]]).astype(np.float32)


# revision 10
# speedup vs baseline: 1.1220x; 1.1220x over previous
"""2-layer GAT (graph attention) on 8 Trainium2 NeuronCores.

Strategy (dst-partitioned, per the 1D graph partitioning scheme):
 - Node tables: G1 = [f=x@W1 | el | er] for all nodes, G2 likewise for layer 1.
   Each core projects a strip of nodes, then AllGather -> full table on
   every core.
 - Per core, dst nodes are split into 128-row blocks. Edges are bucketed by
   (dst block, src-row range) on the host; per-edge rows of the node table
   are fetched with dma_gather (int16 indices force <=32768-row ranges).
 - Edge softmax: ee = exp(leaky_relu(el[src]+er[dst])); aggregation is a
   one-hot matmul: psum[dst,:] += S.T @ [ee*f | ee], where S[e,d]=1{dloc[e]==d}
   is built on-chip with is_equal against an iota row. er[dst] per edge is
   produced without a gather: er_edge = St.T @ er_block where St is the
   transposed one-hot (built from a partition-broadcast copy of dloc).
 - Normalization (divide by sum ee), bias, relu in the block epilogue; the
   layer-2 projection consumes the PE-transposed h tile immediately, so h
   never round-trips through DRAM.
 - dst assignment is chosen so each core's layer-1 dst rows are a prefix of
   its own layer-0 dst rows (er2 comes from the core's own layer-0 epilogue,
   keeping the program SPMD-uniform).
"""

import math
import numpy as np

P = 128

CFG = dict(
    NC=8,
    N0=100000, N1=50000, N2=25000,
    H1=4, D1=64, H2=1, D2=64,
    F0=256, SLOPE=0.2,
    RANGE=32768,
)


def _ceil_to(x, m):
    return -(-x // m) * m


def _derive(cfg):
    NC = cfg["NC"]
    d = {}
    d["n0pc"] = _ceil_to(cfg["N0"], NC * P) // NC
    d["N0P"] = NC * d["n0pc"]
    d["n2pc"] = _ceil_to(cfg["N2"], NC * P) // NC
    d["N2P"] = NC * d["n2pc"]
    rem = cfg["N1"] - d["N2P"]
    assert rem > 0, "layout assumes N1 > padded N2"
    d["bpc"] = _ceil_to(rem, NC * P) // NC
    d["n1pc"] = d["n2pc"] + d["bpc"]
    d["N1P"] = NC * d["n1pc"]
    d["nb0"] = d["n1pc"] // P      # layer-0 dst blocks per core
    d["nb1"] = d["n2pc"] // P      # layer-1 dst blocks per core
    d["nr0"] = -(-d["N0P"] // cfg["RANGE"])
    d["nr1"] = -(-d["N1P"] // cfg["RANGE"])
    d["TW1"] = cfg["F0"] + 64      # 256 f + 4 el + 4 er + pad -> 320 (1280B)
    d["PJ1"] = cfg["F0"] + 2 * cfg["H1"]   # 264 projected cols
    d["TW2"] = 128                 # 64 f2 + el2 + er2 + pad -> 128 (512B)
    d["PJ2"] = cfg["D2"] + 2   # 66: [f2 | el2 | er2]
    d["mk1"] = cfg["F0"] // P      # K chunks for layer-0 projection (2)
    d["mk2"] = (cfg["H1"] * cfg["D1"]) // P  # K chunks for layer-1 proj (2)
    return d


def _l0_owner_local(dst, d):
    """layer-0 dst node -> (core, local row). A-part = first n2pc rows of each
    core (aligned with the layer-1 dst range), B-part = the rest."""
    n2pc, bpc, N2P = d["n2pc"], d["bpc"], d["N2P"]
    a = dst < N2P
    c = np.where(a, dst // n2pc, (dst - N2P) // bpc)
    loc = np.where(a, dst % n2pc, n2pc + (dst - N2P) % bpc)
    return c.astype(np.int64), loc.astype(np.int64)


def _prep_edges(src_rows, dst_c, dst_loc, nb, nr, rng_size, NC):
    """Bucket edges by (core, block, range); pad each bucket to a multiple of
    128 slots, chunk counts maxed across cores (SPMD needs one program).

    Returns (segments, totals, per_core) where
      segments: list of (b, r, local_chunk0, nch, idxcol0) shared by all cores
      totals:   (total_chunks, chunk_base[nb])
      per_core: list of dicts with idx16 [128, S/16], dloc_col [128, C],
                dloc_bc [1, S]
    """
    blk = dst_loc // P
    dl = (dst_loc % P).astype(np.float32)
    rng = src_rows // rng_size
    sl = src_rows % rng_size
    assert sl.max(initial=0) < 32768

    counts = np.zeros((NC, nb, nr), np.int64)
    np.add.at(counts, (dst_c, blk, rng), 1)
    maxch = -(-counts.max(axis=0) // P)          # [nb, nr]
    Cb = maxch.sum(axis=1)                       # [nb]
    chunk_base = np.concatenate([[0], np.cumsum(Cb)])
    total_chunks = int(Cb.sum())
    total_slots = total_chunks * P

    # shared segment table + idx16 column offsets
    segments = []
    idxcol = 0
    seg_info = {}
    for b in range(nb):
        lc = 0
        for r in range(nr):
            nch = int(maxch[b, r])
            if nch == 0:
                continue
            assert nch * P <= 1024, f"gather call too big: {nch * P}"
            segments.append((b, r, lc, nch, idxcol))
            seg_info[(b, r)] = (lc, nch, idxcol)
            lc += nch
            idxcol += nch * 8          # nch*128/16 int16 columns
        assert lc == Cb[b]

    order = np.lexsort((rng, blk, dst_c))
    key = (dst_c * nb + blk) * nr + rng
    skey = key[order]
    bounds = np.searchsorted(skey, np.arange(NC * nb * nr + 1))

    per_core = []
    for c in range(NC):
        idx16 = np.zeros((16, total_slots // 16), np.int16)
        dloc_col = np.full((P, total_chunks), 999.0, np.float32)
        dloc_bc = np.full(total_slots, 999.0, np.float32)
        for b in range(nb):
            for r in range(nr):
                if (b, r) not in seg_info:
                    continue
                lc, nch, col0 = seg_info[(b, r)]
                k = (c * nb + b) * nr + r
                e = order[bounds[k]:bounds[k + 1]]
                n = len(e)
                nslot = nch * P
                assert n <= nslot
                sidx = np.zeros(nslot, np.int64)
                sidx[:n] = sl[e]
                sdl = np.full(nslot, 999.0, np.float32)
                sdl[:n] = dl[e]
                idx16[:, col0:col0 + nch * 8] = (
                    sidx.reshape(-1, 16).T.astype(np.int16))
                g0 = chunk_base[b] + lc
                dloc_col[:, g0:g0 + nch] = sdl.reshape(nch, P).T
                dloc_bc[g0 * P:(g0 + nch) * P] = sdl
        per_core.append(dict(
            idx16=np.tile(idx16, (8, 1)),
            dloc_col=dloc_col,
            dloc_bc=dloc_bc[None, :],
        ))
    return segments, (total_chunks, chunk_base), per_core


def _host_prep(inputs, cfg):
    d = _derive(cfg)
    NC = cfg["NC"]
    F0, H1, D1, D2 = cfg["F0"], cfg["H1"], cfg["D1"], cfg["D2"]

    x = np.asarray(inputs["x"], np.float32)
    xp = np.zeros((d["N0P"], F0), np.float32)
    xp[:cfg["N0"]] = x

    # weight packing: G1 cols = [f | el | er], same for layer 2
    al1 = np.asarray(inputs["al1"], np.float32)
    ar1 = np.asarray(inputs["ar1"], np.float32)
    A_l = np.zeros((H1 * D1, H1), np.float32)
    A_r = np.zeros((H1 * D1, H1), np.float32)
    for h in range(H1):
        A_l[h * D1:(h + 1) * D1, h] = al1[h]
        A_r[h * D1:(h + 1) * D1, h] = ar1[h]
    W1 = np.asarray(inputs["W1"], np.float32)
    W1e = np.concatenate([W1, W1 @ A_l, W1 @ A_r], axis=1)  # [F0, PJ1]

    W2 = np.asarray(inputs["W2"], np.float32)
    al2 = np.asarray(inputs["al2"], np.float32).reshape(-1, 1)
    ar2 = np.asarray(inputs["ar2"], np.float32).reshape(-1, 1)
    W2e = np.concatenate([W2, W2 @ al2, W2 @ ar2], axis=1)  # [256, 66]

    # edges, layer 0: table row of src = src node id
    e0s = np.asarray(inputs["e0_src"], np.int64)
    e0d = np.asarray(inputs["e0_dst"], np.int64)
    c0, loc0 = _l0_owner_local(e0d, d)
    seg0, tot0, pc0 = _prep_edges(e0s, c0, loc0, d["nb0"], d["nr0"],
                                  cfg["RANGE"], NC)

    # edges, layer 1: table row of src node n = owner0(n)*n1pc + local0(n)
    e1s = np.asarray(inputs["e1_src"], np.int64)
    e1d = np.asarray(inputs["e1_dst"], np.int64)
    sc, sloc = _l0_owner_local(e1s, d)
    g2row = sc * d["n1pc"] + sloc
    c1 = e1d // d["n2pc"]
    loc1 = e1d % d["n2pc"]
    seg1, tot1, pc1 = _prep_edges(g2row, c1, loc1, d["nb1"], d["nr1"],
                                  cfg["RANGE"], NC)

    b1 = np.asarray(inputs["b1"], np.float32)
    b2 = np.asarray(inputs["b2"], np.float32)

    consts = dict(
        W1e_a=W1e[:P].copy(), W1e_b=W1e[P:].copy(),
        W2e_a=W2e[:P].copy(), W2e_b=W2e[P:].copy(),
        IOTA_ROW=np.tile(np.arange(P, dtype=np.float32), (P, 1)).copy(),
        IOTA_COL=np.arange(P, dtype=np.float32)[:, None].copy(),
        IDENT=np.eye(P, dtype=np.float32),
        B1T=np.tile(b1[None, :], (P, 1)).copy(),
        B2T=np.tile(b2[None, :], (P, 1)).copy(),
    )

    in_maps = []
    for c in range(NC):
        xT = np.ascontiguousarray(xp[c * d["n0pc"]:(c + 1) * d["n0pc"]].T)
        rows_a = np.arange(c * d["n2pc"], (c + 1) * d["n2pc"])
        rows_b = d["N2P"] + np.arange(c * d["bpc"], (c + 1) * d["bpc"])
        xTd = np.ascontiguousarray(xp[np.concatenate([rows_a, rows_b])].T)
        m = dict(consts)
        m["xT"] = xT
        m["xTd"] = xTd
        m["IDX0"] = pc0[c]["idx16"]
        m["DLC0"] = pc0[c]["dloc_col"]
        m["DLB0"] = pc0[c]["dloc_bc"]
        m["IDX1"] = pc1[c]["idx16"]
        m["DLC1"] = pc1[c]["dloc_col"]
        m["DLB1"] = pc1[c]["dloc_bc"]
        in_maps.append(m)

    meta = dict(d=d, seg0=seg0, tot0=tot0, seg1=seg1, tot1=tot1)
    return in_maps, meta


def _build(cfg, meta, stages="all"):
    import concourse.bass as bass
    import concourse.bacc as bacc
    import concourse.mybir as mybir
    import concourse.tile as tile

    d = meta["d"]
    NC = cfg["NC"]
    F0, H1, D1, D2 = cfg["F0"], cfg["H1"], cfg["D1"], cfg["D2"]
    SLOPE = cfg["SLOPE"]
    TW1, PJ1, TW2, PJ2 = d["TW1"], d["PJ1"], d["TW2"], d["PJ2"]
    nb0, nb1 = d["nb0"], d["nb1"]
    seg0, (C0, cb0) = meta["seg0"], meta["tot0"]
    seg1, (C1, cb1) = meta["seg1"], meta["tot1"]
    S0, S1 = C0 * P, C1 * P
    fdt = mybir.dt.float32
    AL = mybir.AluOpType

    nc = bacc.Bacc("TRN2", target_bir_lowering=False, debug=False,
                   num_devices=NC)

    def din(name, shape, dt=fdt):
        return nc.dram_tensor(name, shape, dt, kind="ExternalInput")

    xT = din("xT", [F0, d["n0pc"]])
    xTd = din("xTd", [F0, d["n1pc"]])
    W1e_a = din("W1e_a", [P, PJ1]); W1e_b = din("W1e_b", [P, PJ1])
    W2e_a = din("W2e_a", [P, PJ2]); W2e_b = din("W2e_b", [P, PJ2])
    IOTA_ROW = din("IOTA_ROW", [P, P]); IOTA_COL = din("IOTA_COL", [P, 1])
    IDENT = din("IDENT", [P, P])
    B1T = din("B1T", [P, F0]); B2T = din("B2T", [P, D2])
    IDX0 = din("IDX0", [P, S0 // 16], mybir.dt.int16)
    DLC0 = din("DLC0", [P, C0]); DLB0 = din("DLB0", [1, S0])
    IDX1 = din("IDX1", [P, S1 // 16], mybir.dt.int16)
    DLC1 = din("DLC1", [P, C1]); DLB1 = din("DLB1", [1, S1])
    OUT = nc.dram_tensor("OUT", [d["n2pc"], D2], fdt, kind="ExternalOutput")

    G1S = nc.dram_tensor("G1S", [d["n0pc"], TW1], fdt)
    G1F = nc.dram_tensor("G1F", [d["N0P"], TW1], fdt, addr_space="Shared")
    G2S = nc.dram_tensor("G2S", [d["n1pc"], TW2], fdt)
    G2F = nc.dram_tensor("G2F", [d["N1P"], TW2], fdt, addr_space="Shared")

    def bcast_row(dram, s0, n):
        """DRAM [1, N] slice -> AP broadcast across 128 partitions."""
        ap = dram[0:1, s0:s0 + n]
        return bass.AP(ap.tensor, ap.offset, [[0, P], [1, n]])

    seg_by_block0 = {}
    for (b, r, lc, nch, col0) in seg0:
        seg_by_block0.setdefault(b, []).append((r, lc, nch, col0))
    seg_by_block1 = {}
    for (b, r, lc, nch, col0) in seg1:
        seg_by_block1.setdefault(b, []).append((r, lc, nch, col0))

    with tile.TileContext(nc) as tc:
        with tc.tile_pool(name="const", bufs=1) as cp, \
             tc.tile_pool(name="work", bufs=2) as wp, \
             tc.tile_pool(name="chk", bufs=4) as kp, \
             tc.tile_pool(name="psum", bufs=2, space="PSUM") as pp, \
             tc.tile_pool(name="psE", bufs=2, space="PSUM") as pe, \
             tc.tile_pool(name="psT", bufs=3, space="PSUM") as pt:

            def const_tile(name, dram, shape, dt=fdt):
                t = cp.tile(shape, dt, tag=name)
                nc.sync.dma_start(out=t[:], in_=dram[:, :])
                return t

            w1a = const_tile("w1a", W1e_a, [P, PJ1])
            w1b = const_tile("w1b", W1e_b, [P, PJ1])
            w2a = const_tile("w2a", W2e_a, [P, PJ2])
            w2b = const_tile("w2b", W2e_b, [P, PJ2])
            iorow = const_tile("iorow", IOTA_ROW, [P, P])
            iocol = const_tile("iocol", IOTA_COL, [P, 1])
            ident = const_tile("ident", IDENT, [P, P])
            b1t = const_tile("b1t", B1T, [P, F0])
            b2t = const_tile("b2t", B2T, [P, D2])
            idx0 = const_tile("idx0", IDX0, [P, S0 // 16], mybir.dt.int16)
            dlc0 = const_tile("dlc0", DLC0, [P, C0])
            idx1 = const_tile("idx1", IDX1, [P, S1 // 16], mybir.dt.int16)
            dlc1 = const_tile("dlc1", DLC1, [P, C1])
            er_sb = cp.tile([P, nb0, H1], fdt, tag="er_sb")
            er2_sb = cp.tile([P, nb1, 1], fdt, tag="er2_sb")

            # ---- layer-0 projection: G1 strip = [f | el | er] ----
            for m in range(d["n0pc"] // P):
                ps = pp.tile([P, PJ1], fdt, tag="agg", space="PSUM")
                for k in range(d["mk1"]):
                    xt = kp.tile([P, P], fdt, tag="xt")
                    nc.sync.dma_start(
                        out=xt[:],
                        in_=xT[k * P:(k + 1) * P, m * P:(m + 1) * P])
                    nc.tensor.matmul(out=ps[:], lhsT=xt[:],
                                     rhs=(w1a if k == 0 else w1b)[:],
                                     start=(k == 0), stop=(k == d["mk1"] - 1))
                sb = kp.tile([P, TW1], fdt, tag="pjsb")
                nc.scalar.copy(out=sb[:, 0:PJ1], in_=ps[:])
                nc.vector.memset(sb[:, PJ1:TW1], 0.0)
                nc.sync.dma_start(out=G1S[m * P:(m + 1) * P, :], in_=sb[:])

            # ---- own-dst er projection (kept in SBUF) ----
            for b in range(nb0):
                ps = pe.tile([P, H1], fdt, tag="er", space="PSUM")
                for k in range(d["mk1"]):
                    xt = kp.tile([P, P], fdt, tag="xt")
                    nc.sync.dma_start(
                        out=xt[:],
                        in_=xTd[k * P:(k + 1) * P, b * P:(b + 1) * P])
                    w = (w1a if k == 0 else w1b)
                    nc.tensor.matmul(out=ps[:], lhsT=xt[:],
                                     rhs=w[:, F0 + H1:F0 + 2 * H1],
                                     start=(k == 0), stop=(k == d["mk1"] - 1))
                nc.scalar.copy(out=er_sb[:, b, :], in_=ps[:])

            nc.gpsimd.collective_compute(
                "AllGather", AL.bypass,
                replica_groups=[list(range(NC))],
                ins=[G1S[:, :]], outs=[G1F[:, :]])

            # ---- layer-0 blocks ----
            for b in range(nb0 if stages != "proj" else 0):
                segs = seg_by_block0.get(b, [])
                Cb = sum(nch for (_, _, nch, _) in segs)
                if Cb == 0:
                    # still must produce zero h -> g2 row
                    Cb = 0
                R = wp.tile([P, max(Cb, 1), TW1], fdt, tag="R")
                for (r, lc, nch, col0) in segs:
                    lo = r * cfg["RANGE"]
                    hi = min(lo + cfg["RANGE"], d["N0P"])
                    nsl = nch * P
                    nc.gpsimd.dma_gather(
                        R[:, lc:lc + nch, :], G1F[lo:hi, :],
                        idx0[:, col0:col0 + nch * 8], nsl, nsl, TW1)
                dlb = wp.tile([P, max(Cb, 1) * P], fdt, tag="dlb")
                if Cb:
                    nc.sync.dma_start(
                        out=dlb[:, :Cb * P],
                        in_=bcast_row(DLB0, int(cb0[b]) * P, Cb * P))
                ps = pp.tile([P, PJ1], fdt, tag="agg", space="PSUM")
                if Cb:
                    gc0 = int(cb0[b])
                    sall = wp.tile([P, Cb, P], fdt, tag="sall")
                    nc.vector.tensor_tensor(
                        out=sall[:],
                        in0=dlc0[:, gc0:gc0 + Cb].unsqueeze(2)
                            .to_broadcast([P, Cb, P]),
                        in1=iorow[:].unsqueeze(1).to_broadcast([P, Cb, P]),
                        op=AL.is_equal)
                    stall = wp.tile([P, Cb, P], fdt, tag="stall")
                    nc.vector.tensor_tensor(
                        out=stall[:],
                        in0=iocol[:].unsqueeze(2).to_broadcast([P, Cb, P]),
                        in1=dlb[:, :Cb * P].rearrange("p (c e) -> p c e", e=P),
                        op=AL.is_equal)
                    erall = pe.tile([P, Cb * H1], fdt, tag="er", space="PSUM")
                    for ci in range(Cb):
                        nc.tensor.matmul(out=erall[:, ci * H1:(ci + 1) * H1],
                                         lhsT=stall[:, ci, :],
                                         rhs=er_sb[:, b, :],
                                         start=True, stop=True)
                    eall = kp.tile([P, Cb, H1], fdt, tag="eall")
                    nc.vector.tensor_tensor(
                        out=eall[:], in0=R[:, :Cb, F0:F0 + H1],
                        in1=erall[:].rearrange("p (c h) -> p c h", h=H1),
                        op=AL.add)
                    e2all = kp.tile([P, Cb, H1], fdt, tag="e2all")
                    nc.vector.tensor_scalar(out=e2all[:], in0=eall[:],
                                            scalar1=SLOPE, scalar2=None,
                                            op0=AL.mult)
                    nc.vector.tensor_tensor(out=e2all[:], in0=eall[:],
                                            in1=e2all[:], op=AL.max)
                    msg = wp.tile([P, Cb, F0 + H1], fdt, tag="msg")
                    nc.scalar.activation(out=msg[:, :, F0:F0 + H1],
                                         in_=e2all[:],
                                         func=mybir.ActivationFunctionType.Exp)
                    nc.vector.tensor_tensor(
                        out=msg[:, :, 0:F0].rearrange(
                            "p c (h j) -> p c h j", h=H1),
                        in0=R[:, :Cb, 0:F0].rearrange(
                            "p c (h j) -> p c h j", h=H1),
                        in1=msg[:, :, F0:F0 + H1].unsqueeze(3)
                            .to_broadcast([P, Cb, H1, D1]),
                        op=AL.mult)
                    for ci in range(Cb):
                        nc.tensor.matmul(out=ps[:, 0:F0 + H1],
                                         lhsT=sall[:, ci, :],
                                         rhs=msg[:, ci, :], start=(ci == 0),
                                         stop=(ci == Cb - 1))
                if Cb == 0:
                    z = kp.tile([P, F0 + H1], fdt, tag="msg")
                    nc.vector.memset(z[:], 0.0)
                    s = kp.tile([P, P], fdt, tag="s")
                    nc.vector.memset(s[:], 0.0)
                    nc.tensor.matmul(out=ps[:, 0:F0 + H1], lhsT=s[:],
                                     rhs=z[:], start=True, stop=True)
                # epilogue: normalize, bias, relu
                rr = kp.tile([P, H1], fdt, tag="rr")
                nc.vector.tensor_scalar(out=rr[:], in0=ps[:, F0:F0 + H1],
                                        scalar1=1e-30, scalar2=None, op0=AL.add)
                nc.vector.reciprocal(out=rr[:], in_=rr[:])
                hsb = wp.tile([P, F0], fdt, tag="hsb")
                nc.vector.tensor_tensor(
                    out=hsb[:].rearrange("p (h j) -> p h j", h=H1),
                    in0=ps[:, 0:F0].rearrange("p (h j) -> p h j", h=H1),
                    in1=rr[:].unsqueeze(2).to_broadcast([P, H1, D1]),
                    op=AL.mult)
                nc.vector.tensor_tensor(out=hsb[:], in0=hsb[:], in1=b1t[:],
                                        op=AL.add)
                nc.scalar.activation(out=hsb[:], in_=hsb[:],
                                     func=mybir.ActivationFunctionType.Relu)
                g2p = pt.tile([P, PJ2], fdt, tag="epi", space="PSUM")
                for k in range(d["mk2"]):
                    tp = pt.tile([P, P], fdt, tag="epi", space="PSUM")
                    nc.tensor.transpose(out=tp[:],
                                        in_=hsb[:, k * P:(k + 1) * P],
                                        identity=ident[:])
                    hT = kp.tile([P, P], fdt, tag="hT")
                    nc.scalar.copy(out=hT[:], in_=tp[:])
                    nc.tensor.matmul(out=g2p[:], lhsT=hT[:],
                                     rhs=(w2a if k == 0 else w2b)[:],
                                     start=(k == 0), stop=(k == d["mk2"] - 1))
                g2sb = kp.tile([P, TW2], fdt, tag="g2sb")
                nc.scalar.copy(out=g2sb[:, 0:PJ2], in_=g2p[:])
                nc.vector.memset(g2sb[:, PJ2:TW2], 0.0)
                nc.sync.dma_start(out=G2S[b * P:(b + 1) * P, :], in_=g2sb[:])
                if b < nb1:
                    nc.scalar.copy(out=er2_sb[:, b, :],
                                   in_=g2sb[:, PJ2 - 1:PJ2])

            if stages in ("all", "nol1g"):
                nc.gpsimd.collective_compute(
                    "AllGather", AL.bypass,
                    replica_groups=[list(range(NC))],
                    ins=[G2S[:, :]], outs=[G2F[:, :]])

            # ---- layer-1 blocks ----
            for b in range(nb1 if stages in ("all", "nol1g") else 0):
                segs = seg_by_block1.get(b, [])
                Cb = sum(nch for (_, _, nch, _) in segs)
                R = wp.tile([P, max(Cb, 1), TW2], fdt, tag="R2")
                if stages == "nol1g":
                    nc.vector.memset(R[:], 0.0)
                else:
                    for (r, lc, nch, col0) in segs:
                        lo = r * cfg["RANGE"]
                        hi = min(lo + cfg["RANGE"], d["N1P"])
                        nsl = nch * P
                        nc.gpsimd.dma_gather(
                            R[:, lc:lc + nch, :], G2F[lo:hi, :],
                            idx1[:, col0:col0 + nch * 8], nsl, nsl, TW2)
                dlb = wp.tile([P, max(Cb, 1) * P], fdt, tag="dlb2")
                if Cb:
                    nc.sync.dma_start(
                        out=dlb[:, :Cb * P],
                        in_=bcast_row(DLB1, int(cb1[b]) * P, Cb * P))
                ps = pp.tile([P, D2 + 1], fdt, tag="agg", space="PSUM")
                if Cb:
                    gc0 = int(cb1[b])
                    sall = wp.tile([P, Cb, P], fdt, tag="sall")
                    nc.vector.tensor_tensor(
                        out=sall[:],
                        in0=dlc1[:, gc0:gc0 + Cb].unsqueeze(2)
                            .to_broadcast([P, Cb, P]),
                        in1=iorow[:].unsqueeze(1).to_broadcast([P, Cb, P]),
                        op=AL.is_equal)
                    stall = wp.tile([P, Cb, P], fdt, tag="stall")
                    nc.vector.tensor_tensor(
                        out=stall[:],
                        in0=iocol[:].unsqueeze(2).to_broadcast([P, Cb, P]),
                        in1=dlb[:, :Cb * P].rearrange("p (c e) -> p c e", e=P),
                        op=AL.is_equal)
                    erall = pe.tile([P, Cb], fdt, tag="er", space="PSUM")
                    for ci in range(Cb):
                        nc.tensor.matmul(out=erall[:, ci:ci + 1],
                                         lhsT=stall[:, ci, :],
                                         rhs=er2_sb[:, b, :],
                                         start=True, stop=True)
                    eall = kp.tile([P, Cb, 1], fdt, tag="eall")
                    nc.vector.tensor_tensor(
                        out=eall[:], in0=R[:, :Cb, D2:D2 + 1],
                        in1=erall[:].unsqueeze(2), op=AL.add)
                    e2all = kp.tile([P, Cb, 1], fdt, tag="e2all")
                    nc.vector.tensor_scalar(out=e2all[:], in0=eall[:],
                                            scalar1=SLOPE, scalar2=None,
                                            op0=AL.mult)
                    nc.vector.tensor_tensor(out=e2all[:], in0=eall[:],
                                            in1=e2all[:], op=AL.max)
                    msg = wp.tile([P, Cb, D2 + 1], fdt, tag="msg2")
                    nc.scalar.activation(out=msg[:, :, D2:D2 + 1],
                                         in_=e2all[:],
                                         func=mybir.ActivationFunctionType.Exp)
                    nc.vector.tensor_tensor(
                        out=msg[:, :, 0:D2],
                        in0=R[:, :Cb, 0:D2],
                        in1=msg[:, :, D2:D2 + 1].to_broadcast([P, Cb, D2]),
                        op=AL.mult)
                    for ci in range(Cb):
                        nc.tensor.matmul(out=ps[:], lhsT=sall[:, ci, :],
                                         rhs=msg[:, ci, :], start=(ci == 0),
                                         stop=(ci == Cb - 1))
                if Cb == 0:
                    z = kp.tile([P, D2 + 1], fdt, tag="msg2")
                    nc.vector.memset(z[:], 0.0)
                    s = kp.tile([P, P], fdt, tag="s")
                    nc.vector.memset(s[:], 0.0)
                    nc.tensor.matmul(out=ps[:], lhsT=s[:], rhs=z[:],
                                     start=True, stop=True)
                rr = kp.tile([P, 1], fdt, tag="rr")
                nc.vector.tensor_scalar(out=rr[:], in0=ps[:, D2:D2 + 1],
                                        scalar1=1e-30, scalar2=None, op0=AL.add)
                nc.vector.reciprocal(out=rr[:], in_=rr[:])
                osb = kp.tile([P, D2], fdt, tag="osb")
                nc.vector.tensor_scalar(out=osb[:], in0=ps[:, 0:D2],
                                        scalar1=rr[:, 0:1], scalar2=None,
                                        op0=AL.mult)
                nc.vector.tensor_tensor(out=osb[:], in0=osb[:], in1=b2t[:],
                                        op=AL.add)
                nc.sync.dma_start(out=OUT[b * P:(b + 1) * P, :], in_=osb[:])

    nc.compile()
    return nc


def kernel(**inputs):
    from concourse import bass_utils
    cfg = CFG
    in_maps, meta = _host_prep(inputs, cfg)
    nc = _build(cfg, meta)
    res = bass_utils.run_bass_kernel_spmd(
        nc, in_maps, core_ids=list(range(cfg["NC"])))
    d = meta["d"]
    out = np.concatenate([res.results[c]["OUT"] for c in range(cfg["NC"])],
                         axis=0)
    return np.ascontiguousarray(out[:cfg["N2"]]).astype(np.float32)
